# revision 1
# baseline (speedup 1.0000x reference)
"""Trainium2 Bass kernel for nn_Discriminator_80195629351349.

Pairwise-column MLP discriminator over k-space columns.

Math (matching the jax reference):
  F[b, w, ch] = |kspace[b, c, h, w]|  (ch = c*H + h)
  Pq = Fq @ W1[:, :CH].T ;  Pa = Fa @ W1[:, CH:].T          [B, W, 18]
  out[b, wi, wc] = sigmoid(W4 @ r3 + b4),  r3 = relu-chain of
                   relu(Pq[wi] + Pa[wc] + b1) through W2, W3
  heat[b, wi] = sum_wc out[b, wi, wc] * cmask[b, wc] / denom[b]
  result[b, h, w] = heat[b, w] if acquiring_mask[b, w] > 0 else 0

Only columns wi with acquiring_mask>0 (16 of 384) contribute to the
output, and the wc sum runs only over [left, right) (191 of 384
columns), so the kernel computes exactly that slice.

Sharding: 8 cores = (batch b in 0..3) x (wc half s in 0..1). Each core
loads its own slice of acquired/acquiring k-space columns (host-side
slicing = the sharding step), computes the column features + all pair
MLP evaluations on-device, and returns partial heat sums [4, NL]
(4 wi-quadrants x NL wi-slots-per-quadrant). Host combines.

On-device layout trick: the 18-channel MLP is packed 4x block-diagonal
across the 128 partitions (quadrant j = partitions 32j..32j+17), so
layers 2-4 are single matmuls with N = NL*NWC <= 512 free columns.
"""

import math
import os

import numpy as np

B, C, H, W = 4, 8, 384, 384
CH = C * H            # 3072 features per column
P = 128               # SBUF partitions
KT = CH // P          # 24 contraction tiles
CHANS = 18            # MLP width
NCORES = 8

# constant-block column layout (one [128, CW] DMA carries everything)
_C_W1AT = 0
_C_W1QT = _C_W1AT + KT * CHANS            # 432
_C_W2 = _C_W1QT + KT * CHANS              # 864
_C_W3 = _C_W2 + P                         # 992
_C_W4 = _C_W3 + P                         # 1120
_C_REP = _C_W4 + 4                        # 1124  quadrant-replication selector
_C_B2 = _C_REP + P                        # 1252
_C_B3 = _C_B2 + 1                         # 1253
_C_B1 = _C_B3 + 1                         # 1254
_C_B4 = _C_B1 + 1                         # 1255
_C_CM = _C_B4 + 1                         # 1256  (+NWC)

_prog_cache: dict = {}
LAST_RESULTS = None   # BassKernelResults of the most recent run (for test.py)


def _build_program(NWC: int, NL: int):
    """Build the SPMD Bass/Tile program for one core.

    NWC: number of wc (acquired) columns this core handles.
    NL:  wi slots per partition-quadrant (total wi slots = 4*NL).
    """
    import concourse.bass as bass
    import concourse.tile as tile
    from concourse import bacc, mybir

    f32 = mybir.dt.float32
    NS = 4 * NL          # wi slots
    NF = NL * NWC        # free columns of the pair block
    CW = _C_CM + NWC
    assert NF <= 512

    nc = bacc.Bacc("TRN2", debug=False)

    # ---- DRAM I/O (per-core shapes; host fills per (b, s)) ----
    aks = nc.dram_tensor("aks", [CH, 2 * NWC], f32, kind="ExternalInput")
    qks = nc.dram_tensor("qks", [CH, 2 * NS], f32, kind="ExternalInput")
    cst = nc.dram_tensor("cst", [P, CW], f32, kind="ExternalInput")
    hp = nc.dram_tensor("hp", [4, NL], f32, kind="ExternalOutput")

    AF = mybir.ActivationFunctionType
    ALU = mybir.AluOpType

    KPC = 6                      # k-tiles per DMA chunk for acquired data
    NCHUNK = KT // KPC           # 4 chunks

    with tile.TileContext(nc) as tc:
        with (
            tc.tile_pool(name="consts", bufs=1) as consts,
            tc.tile_pool(name="adata", bufs=NCHUNK) as adata,
            tc.tile_pool(name="qdata", bufs=1) as qdata,
            tc.tile_pool(name="feat", bufs=1) as feat,
            tc.tile_pool(name="sq", bufs=3) as sqp,
            tc.tile_pool(name="m2", bufs=3) as m2p,
            tc.tile_pool(name="mlp", bufs=1) as mlp,
            tc.tile_pool(name="psA", bufs=1, space="PSUM") as psA,
            tc.tile_pool(name="psB", bufs=1, space="PSUM") as psB,
        ):
            # ---- q k-space first on the scalar DMA queue (tiny, feeds the
            # head of the DVE pipeline), then the constant block ----
            qks_r = qks[:].rearrange("(k p) n -> p k n", p=P)
            qc = qdata.tile([P, KT * 2 * NS], f32, tag="qc")
            nc.scalar.dma_start(out=qc.rearrange("p (k n) -> p k n", k=KT),
                                in_=qks_r)
            cst_s = consts.tile([P, CW], f32, tag="cst")
            nc.scalar.dma_start(out=cst_s, in_=cst[:])
            w2bd_s = cst_s[:, _C_W2:_C_W2 + P]
            w3bd_s = cst_s[:, _C_W3:_C_W3 + P]
            w4bd_s = cst_s[:, _C_W4:_C_W4 + 4]
            b2_s = cst_s[:, _C_B2:_C_B2 + 1]
            b3_s = cst_s[:, _C_B3:_C_B3 + 1]
            b1_s = cst_s[0:CHANS, _C_B1:_C_B1 + 1]
            b4_s = cst_s[0:4, _C_B4:_C_B4 + 1]
            cm_s = cst_s[0:4, _C_CM:_C_CM + NWC]

            # ---- q path first: small and cheap, frees PE to start early ----
            fq_s = feat.tile([P, KT, NS], f32, tag="fq")
            sqq = sqp.tile([P, KT * 2 * NS], f32, tag="sqq")
            nc.vector.tensor_mul(sqq, qc, qc)
            vq = sqq.rearrange("p (n r) -> p n r", r=2)
            m2q = m2p.tile([P, KT * NS], f32, tag="m2q")
            nc.vector.tensor_add(m2q, vq[:, :, 0], vq[:, :, 1])
            nc.scalar.sqrt(fq_s.rearrange("p k n -> p (k n)"), m2q)

            psum_pq = psA.tile([CHANS, NS], f32, tag="ppq")
            for k in range(KT):
                nc.tensor.matmul(
                    out=psum_pq,
                    lhsT=cst_s[:, _C_W1QT + k * CHANS:_C_W1QT + (k + 1) * CHANS],
                    rhs=fq_s[:, k, :],
                    start=(k == 0),
                    stop=(k == KT - 1),
                )
            pq_s = mlp.tile([CHANS, NS], f32, tag="pq")
            nc.vector.tensor_copy(pq_s, psum_pq)
            pq4 = mlp.tile([P, NL], f32, tag="pq4")
            nc.vector.memset(pq4, 0.0)
            for j in range(4):
                eng = nc.scalar if j % 2 == 0 else nc.sync
                eng.dma_start(out=pq4[32 * j:32 * j + CHANS, :],
                              in_=pq_s[:, j * NL:(j + 1) * NL])

            # ---- stream acquired k-space; ch = k*128 + p ----
            aks_r = aks[:].rearrange("(k p) n -> p k n", p=P)
            achunks = []
            for i in range(NCHUNK):
                ac = adata.tile([P, KPC * 2 * NWC], f32, tag=f"ac{i}")
                eng = nc.sync if i % 2 == 0 else nc.scalar
                eng.dma_start(
                    out=ac.rearrange("p (k n) -> p k n", k=KPC),
                    in_=aks_r[:, i * KPC:(i + 1) * KPC, :])
                achunks.append(ac)

            # ---- |z| = sqrt(re^2 + im^2), one fat op per chunk ----
            fa_s = feat.tile([P, KT, NWC], f32, tag="fa")
            fa_flat = fa_s.rearrange("p k n -> p (k n)")
            psum_pa = psA.tile([CHANS, NWC], f32, tag="ppa")
            for i in range(NCHUNK):
                src = achunks[i]
                sqt = sqp.tile([P, KPC * 2 * NWC], f32, tag="sqa")
                nc.vector.tensor_mul(sqt, src, src)
                v = sqt.rearrange("p (n r) -> p n r", r=2)
                m2 = m2p.tile([P, KPC * NWC], f32, tag="m2a")
                nc.vector.tensor_add(m2, v[:, :, 0], v[:, :, 1])
                nc.scalar.sqrt(
                    fa_flat[:, i * KPC * NWC:(i + 1) * KPC * NWC], m2)
                for k in range(i * KPC, (i + 1) * KPC):
                    nc.tensor.matmul(
                        out=psum_pa,
                        lhsT=cst_s[:, _C_W1AT + k * CHANS:
                                   _C_W1AT + (k + 1) * CHANS],
                        rhs=fa_s[:, k, :],
                        start=(k == 0),
                        stop=(k == KT - 1),
                    )

            # prefetch the sigmoid ACT table right after the last a-sqrt so
            # the real sigmoid doesn't pay the 1.3us table swap on the tail
            sigdummy = mlp.tile([1, 1], f32, tag="sigd")
            nc.scalar.activation(out=sigdummy, in_=fa_s[0:1, KT - 1, 0:1],
                                 func=AF.Sigmoid)

            # Pa + b1 on DVE (keeps ACT free for the sigmoid table)
            pa_s = mlp.tile([CHANS, NWC], f32, tag="pa")
            nc.vector.tensor_scalar(out=pa_s, in0=psum_pa, scalar1=b1_s,
                                    scalar2=None, op0=ALU.add)
            # replicate into the 4 partition quadrants with one matmul
            # against a constant selector (pad rows come out zero)
            pa4 = psA.tile([P, NWC], f32, tag="pa4")
            nc.tensor.matmul(out=pa4, lhsT=cst_s[0:CHANS, _C_REP:_C_REP + P],
                             rhs=pa_s, start=True, stop=True)

            # ---- pair MLP, pipelined as two wi-slot halves so PE matmuls
            # of one half overlap DVE relus of the other ----
            NLH = max(NL // 2, 1)
            halves = [(0, NLH), (NLH, NL)] if NL > 1 else [(0, 1)]
            h1p = mlp.tile([P, NF], f32, tag="h1p")
            hp_s = mlp.tile([4, NL], f32, tag="hps")
            scr = mlp.tile([4, NL, NWC], f32, tag="scr")
            for hi, (l0, l1) in enumerate(halves):
                HF = (l1 - l0) * NWC
                off = l0 * NWC
                h1h = h1p[:, off:off + HF]
                for lw in range(l0, l1):
                    nc.vector.tensor_scalar(
                        out=h1p[:, lw * NWC:(lw + 1) * NWC],
                        in0=pa4,
                        scalar1=pq4[:, lw:lw + 1],
                        scalar2=0.0,
                        op0=ALU.add,
                        op1=ALU.max,
                    )
                psum2 = psB.tile([P, HF], f32, tag=f"ps2_{hi}")
                nc.tensor.matmul(out=psum2, lhsT=w2bd_s, rhs=h1h,
                                 start=True, stop=True)
                h2p = mlp.tile([P, HF], f32, tag=f"h2p_{hi}")
                nc.vector.tensor_scalar(out=h2p, in0=psum2, scalar1=b2_s,
                                        scalar2=0.0, op0=ALU.add, op1=ALU.max)
                psum3 = psB.tile([P, HF], f32, tag=f"ps3_{hi}")
                nc.tensor.matmul(out=psum3, lhsT=w3bd_s, rhs=h2p,
                                 start=True, stop=True)
                h3p = mlp.tile([P, HF], f32, tag=f"h3p_{hi}")
                nc.vector.tensor_scalar(out=h3p, in0=psum3, scalar1=b3_s,
                                        scalar2=0.0, op0=ALU.add, op1=ALU.max)
                psum4 = psB.tile([4, HF], f32, tag="ps4")
                nc.tensor.matmul(out=psum4, lhsT=w4bd_s, rhs=h3p,
                                 start=True, stop=True)
                sig = mlp.tile([4, HF], f32, tag=f"sig_{hi}")
                nc.scalar.activation(out=sig, in_=psum4, func=AF.Sigmoid,
                                     bias=b4_s, scale=1.0)
                for lw in range(l0, l1):
                    nc.vector.tensor_mul(
                        scr[:, lw, :],
                        sig[:, (lw - l0) * NWC:(lw - l0 + 1) * NWC], cm_s)

            # ---- heat[j, lw] = sum_c sig[j, lw*NWC+c] * cm[c] ----
            nc.vector.reduce_sum(hp_s, scr, axis=mybir.AxisListType.X)
            nc.sync.dma_start(out=hp[:], in_=hp_s)

    nc.finalize()
    return nc


def _run_sim(nc, in_maps):
    """CoreSim (CPU instruction simulator) path for local dev testing."""
    from concourse.bass_interp import MultiCoreSim
    from concourse.bass_utils import BassKernelResults

    sim = MultiCoreSim(nc, num_cores=len(in_maps))
    for core_id, core in sim.cores.items():
        for name, arr in in_maps[core_id].items():
            core.tensor(name)[:] = arr
    sim.simulate()
    results = [
        {"hp": np.array(sim.cores[i].tensor("hp"))} for i in range(len(in_maps))
    ]
    return BassKernelResults(results=results, instructions_and_trace=None,
                             profile_json=None, exec_time_ns=None)


def _mask_geometry(acquired_mask, acquiring_mask):
    """Replicates the reference's left/right/cmask/denom logic exactly."""
    am = np.asarray(acquired_mask, np.float32)
    qm = np.asarray(acquiring_mask, np.float32)
    mid = W // 2
    right = mid + np.argmax(am[:, mid:] < 1.0, axis=1)
    left = np.argmax(am[:, :mid][:, ::-1] < 1.0, axis=1) + 1
    cols = np.arange(W)
    cmask = (cols[None, :] >= left[:, None]) & (cols[None, :] < right[:, None])
    denom = (right - left).astype(np.float32)
    active = [np.nonzero(qm[b] > 0)[0] for b in range(B)]
    return left.astype(int), right.astype(int), cmask, denom, active


def kernel(acquired_kspace, acquiring_kspace, acquired_mask, acquiring_mask,
           W1, b1, W2, b2, W3, b3, W4, b4):
    global LAST_RESULTS
    from concourse.bass_utils import run_bass_kernel_spmd

    acquired_kspace = np.ascontiguousarray(np.asarray(acquired_kspace, np.float32))
    acquiring_kspace = np.ascontiguousarray(np.asarray(acquiring_kspace, np.float32))
    W1 = np.asarray(W1, np.float32)
    b1 = np.asarray(b1, np.float32)
    W2 = np.asarray(W2, np.float32)
    b2 = np.asarray(b2, np.float32)
    W3 = np.asarray(W3, np.float32)
    b3 = np.asarray(b3, np.float32)
    W4 = np.asarray(W4, np.float32)
    b4 = np.asarray(b4, np.float32)

    left, right, cmask, denom, active = _mask_geometry(acquired_mask, acquiring_mask)

    nmax = max(len(a) for a in active)
    out = np.zeros((B, H, W), np.float32)
    if nmax == 0:
        return out

    span = max(int((right - left).max()), 1)
    NL = max(1, math.ceil(nmax / 4))          # wi slots per quadrant
    NWC = max(1, math.ceil(span / 2))         # wc columns per core
    NS = 4 * NL
    assert NL * NWC <= 512, (NL, NWC)

    # ---- shared constant block [128, CW] ----
    CW = _C_CM + NWC
    W1q, W1a = W1[:, :CH], W1[:, CH:]
    cstv = np.zeros((P, CW), np.float32)
    cstv[:, _C_W1AT:_C_W1AT + KT * CHANS] = (
        W1a.T.reshape(KT, P, CHANS).transpose(1, 0, 2).reshape(P, KT * CHANS))
    cstv[:, _C_W1QT:_C_W1QT + KT * CHANS] = (
        W1q.T.reshape(KT, P, CHANS).transpose(1, 0, 2).reshape(P, KT * CHANS))
    for j in range(4):
        sl = slice(32 * j, 32 * j + CHANS)
        cstv[sl, _C_W2 + 32 * j:_C_W2 + 32 * j + CHANS] = W2.T
        cstv[sl, _C_W3 + 32 * j:_C_W3 + 32 * j + CHANS] = W3.T
        cstv[sl, _C_W4 + j] = W4[0]
        cstv[sl, _C_B2] = b2
        cstv[sl, _C_B3] = b3
        # selector: lhsT rows i, cols 32j+i -> replicates [18, n] into quads
        cstv[:CHANS, _C_REP + 32 * j:_C_REP + 32 * j + CHANS] = np.eye(
            CHANS, dtype=np.float32)
    cstv[:CHANS, _C_B1] = b1
    cstv[:4, _C_B4] = float(b4[0])

    # ---- per-core slices ----
    in_maps = []
    meta = []
    for b in range(B):
        aw = active[b]
        awp = np.zeros(NS, np.int64)
        if len(aw):
            awp[:len(aw)] = aw
            awp[len(aw):] = aw[0]
        # acquiring features for the active wi columns: [CH, 2*NS]
        qks = np.ascontiguousarray(
            acquiring_kspace[b][:, :, awp, :].reshape(CH, 2 * NS))
        for s in range(2):
            w0 = int(left[b]) + s * NWC
            w1e = max(min(w0 + NWC, W), w0)
            buf = np.zeros((C, H, NWC, 2), np.float32)
            cstc = cstv.copy()
            if w0 < W and w1e > w0:
                buf[:, :, :w1e - w0, :] = acquired_kspace[b, :, :, w0:w1e, :]
                d = denom[b] if denom[b] != 0 else 1.0
                cstc[:4, _C_CM:_C_CM + (w1e - w0)] = (
                    cmask[b, w0:w1e].astype(np.float32) / d)[None, :]
            aks = buf.reshape(CH, 2 * NWC)
            in_maps.append(dict(aks=aks, qks=qks, cst=cstc))
            meta.append((b, s))

    key = (NWC, NL)
    if key not in _prog_cache:
        _prog_cache[key] = _build_program(NWC, NL)
    nc = _prog_cache[key]

    trace = bool(int(os.environ.get("CABSK_TRACE", "0")))
    tmpdir = os.environ.get("CABSK_TMPDIR") or None
    if tmpdir:
        import tempfile
        tmpdir = tempfile.mkdtemp(dir=tmpdir)
    if os.environ.get("CABSK_SIM", "0") == "1":
        res = _run_sim(nc, in_maps)
    else:
        res = run_bass_kernel_spmd(nc, in_maps, core_ids=list(range(NCORES)),
                                   trace=trace, tmpdir=tmpdir)
    LAST_RESULTS = res

    heat = np.zeros((B, W), np.float32)
    for ci, (b, s) in enumerate(meta):
        hpv = res.results[ci]["hp"]          # [4, NL]
        aw = active[b]
        for t in range(len(aw)):
            heat[b, aw[t]] += hpv[t // NL, t % NL]
    out[:] = heat[:, None, :]
    return out



# revision 4
# speedup vs baseline: 1.3227x; 1.3227x over previous
"""Trainium2 Bass kernel for nn_Discriminator_80195629351349.

Pairwise-column MLP discriminator over k-space columns.

Math (matching the jax reference):
  F[b, w, ch] = |kspace[b, c, h, w]|  (ch = c*H + h)
  Pq = Fq @ W1[:, :CH].T ;  Pa = Fa @ W1[:, CH:].T          [B, W, 18]
  out[b, wi, wc] = sigmoid(W4 @ r3 + b4),  r3 = relu-chain of
                   relu(Pq[wi] + Pa[wc] + b1) through W2, W3
  heat[b, wi] = sum_wc out[b, wi, wc] * cmask[b, wc] / denom[b]
  result[b, h, w] = heat[b, w] if acquiring_mask[b, w] > 0 else 0

Only columns wi with acquiring_mask>0 (16 of 384) contribute to the
output, and the wc sum runs only over [left, right) (191 of 384
columns), so the kernel computes exactly that slice.

Sharding: 8 cores = (batch b in 0..3) x (wc half s in 0..1). Each core
gets a host-packed bf16 block of its 96 acquired columns PLUS the 16
acquiring columns (one merged 112-column stream), computes column
features + all pair MLP evaluations on-device, and returns partial heat
sums [4, NL] (4 wi-quadrants x NL wi-slots-per-quadrant). Host combines.

Device pipeline (all matmuls bf16, 1 cyc/col vs 4 for fp32):
  - ACT tables (sqrt + sigmoid) are pre-loaded via dummy activations at
    kernel start so no 1.3us table swap lands mid-pipeline.
  - X data arrives partition-major in 4 chunks (contiguous DMA, re/im
    as packed blocks so DVE square/add run in 2x/4x perf modes).
  - per chunk: DVE square -> DVE add -> ACT sqrt -> 6x PE matmul with a
    merged [128, 36] W1 tile ([W1a | W1q]) accumulating PSUM [36, 112].
  - junction: extract Pa+b1 / Pq (DVE), one replicate-matmul against a
    block-diagonal selector -> PSUM [128, 112] holding quadrant-
    replicated Pa and Pq; 4 tiny DVE copies build the pq4 column table.
  - pair MLP in two wi-halves: DVE relu-builds, PE block-diag W2/W3/W4,
    ACT relu (bias fused; relu is in every ACT table so it needs no
    extra table load), ACT sigmoid (bias fused), DVE cmask-mul.
  - DVE reduce -> hp [4, NL] -> DMA out.
"""

import math
import os

import numpy as np
import ml_dtypes

BF16 = np.dtype(ml_dtypes.bfloat16)

B, C, H, W = 4, 8, 384, 384
CH = C * H            # 3072 features per column
P = 128               # SBUF partitions
KT = CH // P          # 24 contraction tiles
CHANS = 18            # MLP width
NCORES = 8
KPC = 6               # k-tiles per DMA chunk
NCHUNK = KT // KPC    # 4 chunks

# cstb (bf16 const block) column layout. The merged W1 lhsT tile per k is
# [W1a (18) | pad (14) | W1q (18)] so Pq lands at PSUM partitions 32:50
# (engine APs must start at a multiple of 32).
MW = 50
_C_W1 = 0                          # KT * MW merged W1 tiles
_C_REP = _C_W1 + KT * MW           # 1200: quadrant-replication selector
_C_W2 = _C_REP + P
_C_W3 = _C_W2 + P
_C_W4 = _C_W3 + P
CBW = _C_W4 + 4

_prog_cache: dict = {}
LAST_RESULTS = None   # BassKernelResults of the most recent run (for test.py)


def _build_program(NWC: int, NL: int):
    """Build the SPMD Bass/Tile program for one core.

    NWC: number of wc (acquired) columns this core handles.
    NL:  wi slots per partition-quadrant (total wi slots = 4*NL).
    """
    import concourse.bass as bass
    import concourse.tile as tile
    from concourse import bacc, mybir

    f32 = mybir.dt.float32
    bf16 = mybir.dt.bfloat16
    NS = 4 * NL           # wi slots
    NCOL = NWC + NS       # merged stream columns (acquired + acquiring)
    NF = NL * NWC         # free columns of the pair block
    assert NF <= 512

    nc = bacc.Bacc("TRN2", debug=False)

    AF = mybir.ActivationFunctionType
    ALU = mybir.AluOpType

    # ---- DRAM I/O (per-core shapes; host fills per (b, s)) ----
    xd = nc.dram_tensor("xd", [P, KT * 2 * NCOL], bf16, kind="ExternalInput")
    cb = nc.dram_tensor("cb", [P, CBW], bf16, kind="ExternalInput")
    cf = nc.dram_tensor("cf", [P, 4], f32, kind="ExternalInput")
    cmt = nc.dram_tensor("cmt", [4, NF], f32, kind="ExternalInput")
    hp = nc.dram_tensor("hp", [4, NL], f32, kind="ExternalOutput")

    with tile.TileContext(nc) as tc:
        with (
            tc.tile_pool(name="consts", bufs=1) as consts,
            tc.tile_pool(name="xdata", bufs=1) as xpool,
            tc.tile_pool(name="feat", bufs=1) as feat,
            tc.tile_pool(name="sq", bufs=2) as sqp,
            tc.tile_pool(name="m2", bufs=2) as m2p,
            tc.tile_pool(name="mlp", bufs=1) as mlp,
            tc.tile_pool(name="psA", bufs=1, space="PSUM") as psA,
            tc.tile_pool(name="psB", bufs=1, space="PSUM") as psB,
        ):
            # ---- preload both ACT tables (sigmoid, sqrt) before any real
            # activation so the 1.3us loads hide under the DMA window ----
            dum = mlp.tile([1, 4], f32, tag="dum")
            nc.vector.memset(dum[:, 0:1], 0.25)
            nc.scalar.activation(out=dum[:, 1:2], in_=dum[:, 0:1],
                                 func=AF.Sigmoid)
            nc.scalar.activation(out=dum[:, 2:3], in_=dum[:, 0:1],
                                 func=AF.Sqrt)

            # ---- constants: big bf16 block on SP, small fp32 on gpsimd ----
            cb_s = consts.tile([P, CBW], bf16, tag="cb")
            nc.sync.dma_start(out=cb_s, in_=cb[:])
            cf_s = consts.tile([P, 4], f32, tag="cf")
            nc.gpsimd.dma_start(out=cf_s, in_=cf[:])
            cm_s = consts.tile([4, NF], f32, tag="cm")
            nc.gpsimd.dma_start(out=cm_s, in_=cmt[:])

            b1c = cf_s[0:CHANS, 0:1]
            b2c = cf_s[:, 1:2]
            b3c = cf_s[:, 2:3]
            b4c = cf_s[0:4, 3:4]

            # ---- X chunks: contiguous [128, KPC*2*NCOL] bf16 DMAs ----
            CW = KPC * 2 * NCOL
            xchunks = []
            for i in range(NCHUNK):
                xch = xpool.tile([P, CW], bf16, tag=f"x{i}")
                eng = nc.sync if i % 2 == 0 else nc.gpsimd
                eng.dma_start(out=xch, in_=xd[:][:, i * CW:(i + 1) * CW])
                xchunks.append(xch)

            # ---- per chunk: square, add (re/im packed blocks), sqrt,
            # and 6 accumulating matmuls with merged [128, 36] weights ----
            F = feat.tile([P, KT, NCOL], bf16, tag="F")
            psumP = psA.tile([MW, NCOL], f32, tag="pP")
            for i in range(NCHUNK):
                sq = sqp.tile([P, CW], bf16, tag=f"sq{i % 2}")
                nc.vector.tensor_mul(sq, xchunks[i], xchunks[i])
                sqv = sq.rearrange("p (k r n) -> p k r n", k=KPC, r=2)
                m2 = m2p.tile([P, KPC, NCOL], bf16, tag=f"m2{i % 2}")
                nc.vector.tensor_add(m2, sqv[:, :, 0, :], sqv[:, :, 1, :])
                nc.scalar.sqrt(F[:, i * KPC:(i + 1) * KPC, :], m2)
                for k in range(i * KPC, (i + 1) * KPC):
                    nc.tensor.matmul(
                        out=psumP,
                        lhsT=cb_s[:, _C_W1 + k * MW:_C_W1 + (k + 1) * MW],
                        rhs=F[:, k, :],
                        start=(k == 0),
                        stop=(k == KT - 1),
                    )

            # ---- junction: Pa+b1 / Pq extract, quadrant replicate ----
            paq = mlp.tile([CHANS, NCOL], bf16, tag="paq")
            nc.vector.tensor_scalar(out=paq[:, 0:NWC],
                                    in0=psumP[0:CHANS, 0:NWC],
                                    scalar1=b1c, scalar2=None, op0=ALU.add)
            nc.vector.tensor_copy(paq[:, NWC:NCOL],
                                  psumP[32:32 + CHANS, NWC:NCOL])
            psumR = psA.tile([P, NCOL], f32, tag="pR")
            nc.tensor.matmul(out=psumR, lhsT=cb_s[0:CHANS, _C_REP:_C_REP + P],
                             rhs=paq, start=True, stop=True)
            pa4 = mlp.tile([P, NWC], bf16, tag="pa4")
            nc.vector.tensor_copy(pa4, psumR[:, 0:NWC])
            pq4 = mlp.tile([P, NL], f32, tag="pq4")
            for j in range(4):
                nc.vector.tensor_copy(
                    pq4[32 * j:32 * (j + 1), :],
                    psumR[32 * j:32 * (j + 1), NWC + j * NL:NWC + (j + 1) * NL])

            # ---- pair MLP, two wi-halves pipelined across DVE/PE/ACT ----
            NLH = max(NL // 2, 1)
            halves = [(0, NLH), (NLH, NL)] if NL > 1 else [(0, 1)]
            h1 = mlp.tile([P, NF], bf16, tag="h1")
            scr = mlp.tile([4, NL, NWC], f32, tag="scr")
            w2bd = cb_s[:, _C_W2:_C_W2 + P]
            w3bd = cb_s[:, _C_W3:_C_W3 + P]
            w4bd = cb_s[:, _C_W4:_C_W4 + 4]
            for hi, (l0, l1) in enumerate(halves):
                HF = (l1 - l0) * NWC
                for lw in range(l0, l1):
                    nc.vector.tensor_scalar(
                        out=h1[:, lw * NWC:(lw + 1) * NWC],
                        in0=pa4,
                        scalar1=pq4[:, lw:lw + 1],
                        scalar2=0.0,
                        op0=ALU.add,
                        op1=ALU.max,
                    )
                ps2 = psB.tile([P, HF], f32, tag=f"ps2_{hi}")
                nc.tensor.matmul(out=ps2, lhsT=w2bd,
                                 rhs=h1[:, l0 * NWC:l1 * NWC],
                                 start=True, stop=True)
                h2 = mlp.tile([P, HF], bf16, tag=f"h2_{hi}")
                nc.scalar.activation(out=h2, in_=ps2, func=AF.Relu,
                                     bias=b2c, scale=1.0)
                ps3 = psB.tile([P, HF], f32, tag=f"ps3_{hi}")
                nc.tensor.matmul(out=ps3, lhsT=w3bd, rhs=h2,
                                 start=True, stop=True)
                h3 = mlp.tile([P, HF], bf16, tag=f"h3_{hi}")
                nc.scalar.activation(out=h3, in_=ps3, func=AF.Relu,
                                     bias=b3c, scale=1.0)
                ps4 = psB.tile([4, HF], f32, tag=f"ps4_{hi}")
                nc.tensor.matmul(out=ps4, lhsT=w4bd, rhs=h3,
                                 start=True, stop=True)
                sig = mlp.tile([4, HF], f32, tag=f"sig_{hi}")
                nc.scalar.activation(out=sig, in_=ps4, func=AF.Sigmoid,
                                     bias=b4c, scale=1.0)
                nc.vector.tensor_mul(
                    scr[:, l0:l1, :].rearrange("q l c -> q (l c)"),
                    sig, cm_s[:, l0 * NWC:l1 * NWC])

            # ---- heat[j, lw] = sum_c scr[j, lw, c] ----
            hp_s = mlp.tile([4, NL], f32, tag="hps")
            nc.vector.reduce_sum(hp_s, scr, axis=mybir.AxisListType.X)
            nc.sync.dma_start(out=hp[:], in_=hp_s)

    nc.finalize()
    return nc


def _run_sim(nc, in_maps):
    """CoreSim (CPU instruction simulator) path for local dev testing."""
    from concourse.bass_interp import MultiCoreSim
    from concourse.bass_utils import BassKernelResults

    sim = MultiCoreSim(nc, num_cores=len(in_maps))
    for core_id, core in sim.cores.items():
        for name, arr in in_maps[core_id].items():
            core.tensor(name)[:] = arr
    sim.simulate()
    results = [
        {"hp": np.array(sim.cores[i].tensor("hp"))} for i in range(len(in_maps))
    ]
    return BassKernelResults(results=results, instructions_and_trace=None,
                             profile_json=None, exec_time_ns=None)


def _mask_geometry(acquired_mask, acquiring_mask):
    """Replicates the reference's left/right/cmask/denom logic exactly."""
    am = np.asarray(acquired_mask, np.float32)
    qm = np.asarray(acquiring_mask, np.float32)
    mid = W // 2
    right = mid + np.argmax(am[:, mid:] < 1.0, axis=1)
    left = np.argmax(am[:, :mid][:, ::-1] < 1.0, axis=1) + 1
    cols = np.arange(W)
    cmask = (cols[None, :] >= left[:, None]) & (cols[None, :] < right[:, None])
    denom = (right - left).astype(np.float32)
    active = [np.nonzero(qm[b] > 0)[0] for b in range(B)]
    return left.astype(int), right.astype(int), cmask, denom, active


def kernel(acquired_kspace, acquiring_kspace, acquired_mask, acquiring_mask,
           W1, b1, W2, b2, W3, b3, W4, b4):
    global LAST_RESULTS
    from concourse.bass_utils import run_bass_kernel_spmd

    acquired_kspace = np.asarray(acquired_kspace, np.float32)
    acquiring_kspace = np.asarray(acquiring_kspace, np.float32)
    W1 = np.asarray(W1, np.float32)
    b1 = np.asarray(b1, np.float32)
    W2 = np.asarray(W2, np.float32)
    b2 = np.asarray(b2, np.float32)
    W3 = np.asarray(W3, np.float32)
    b3 = np.asarray(b3, np.float32)
    W4 = np.asarray(W4, np.float32)
    b4 = np.asarray(b4, np.float32)

    left, right, cmask, denom, active = _mask_geometry(acquired_mask, acquiring_mask)

    nmax = max(len(a) for a in active)
    out = np.zeros((B, H, W), np.float32)
    if nmax == 0:
        return out

    span = max(int((right - left).max()), 1)
    NL = max(1, math.ceil(nmax / 4))          # wi slots per quadrant
    NWC = max(1, math.ceil(span / 2))         # wc columns per core
    NS = 4 * NL
    NCOL = NWC + NS
    NF = NL * NWC
    assert NF <= 512, (NL, NWC)

    # ---- shared bf16 constant block [128, CBW] ----
    W1q, W1a = W1[:, :CH], W1[:, CH:]
    cbv = np.zeros((P, CBW), np.float32)
    # merged per-k lhsT tiles: [W1a_k (18) | W1q_k (18)]
    m = np.zeros((KT, P, MW), np.float32)
    m[:, :, :CHANS] = W1a.T.reshape(KT, P, CHANS)
    m[:, :, 32:32 + CHANS] = W1q.T.reshape(KT, P, CHANS)
    cbv[:, _C_W1:_C_W1 + KT * MW] = (
        m.transpose(1, 0, 2).reshape(P, KT * MW))
    for j in range(4):
        sl = slice(32 * j, 32 * j + CHANS)
        # selector: lhsT rows ch, cols 32j+ch -> replicates [18, n] to quads
        cbv[:CHANS, _C_REP + 32 * j:_C_REP + 32 * j + CHANS] = np.eye(
            CHANS, dtype=np.float32)
        cbv[sl, _C_W2 + 32 * j:_C_W2 + 32 * j + CHANS] = W2.T
        cbv[sl, _C_W3 + 32 * j:_C_W3 + 32 * j + CHANS] = W3.T
        cbv[sl, _C_W4 + j] = W4[0]
    cbv = cbv.astype(BF16)

    # ---- shared fp32 bias columns [128, 4] ----
    cfv = np.zeros((P, 4), np.float32)
    cfv[:CHANS, 0] = b1
    for j in range(4):
        sl = slice(32 * j, 32 * j + CHANS)
        cfv[sl, 1] = b2
        cfv[sl, 2] = b3
    cfv[:4, 3] = float(b4[0])

    # ---- per-core slices ----
    in_maps = []
    meta = []
    for b in range(B):
        aw = active[b]
        awp = np.zeros(NS, np.int64)
        if len(aw):
            awp[:len(aw)] = aw
            awp[len(aw):] = aw[0]
        # acquiring features for the active wi columns: [CH, 16, 2]
        qcols = acquiring_kspace[b].reshape(CH, W, 2)[:, awp, :]
        for s in range(2):
            w0 = int(left[b]) + s * NWC
            w1e = max(min(w0 + NWC, W), w0)
            nv = w1e - w0
            xf = np.zeros((CH, NWC + NS, 2), np.float32)
            if nv > 0:
                xf[:, :nv, :] = acquired_kspace[b].reshape(CH, W, 2)[:, w0:w1e, :]
            xf[:, NWC:, :] = qcols
            # -> [p, k, r, n] partition-major, re/im as packed blocks
            xv = (xf.transpose(0, 2, 1).reshape(KT, P, 2, NCOL)
                  .transpose(1, 0, 2, 3).reshape(P, KT * 2 * NCOL))
            cmv = np.zeros((4, NF), np.float32)
            if nv > 0:
                d = denom[b] if denom[b] != 0 else 1.0
                row = (cmask[b, w0:w1e].astype(np.float32) / d)
                for lw in range(NL):
                    cmv[:, lw * NWC:lw * NWC + nv] = row[None, :]
            in_maps.append(dict(xd=np.ascontiguousarray(xv.astype(BF16)),
                                cb=cbv, cf=cfv, cmt=cmv))
            meta.append((b, s))

    key = (NWC, NL)
    if key not in _prog_cache:
        _prog_cache[key] = _build_program(NWC, NL)
    nc = _prog_cache[key]

    trace = bool(int(os.environ.get("CABSK_TRACE", "0")))
    tmpdir = os.environ.get("CABSK_TMPDIR") or None
    if tmpdir:
        import tempfile
        tmpdir = tempfile.mkdtemp(dir=tmpdir)
    if os.environ.get("CABSK_SIM", "0") == "1":
        res = _run_sim(nc, in_maps)
    else:
        res = run_bass_kernel_spmd(nc, in_maps, core_ids=list(range(NCORES)),
                                   trace=trace, tmpdir=tmpdir)
    LAST_RESULTS = res

    heat = np.zeros((B, W), np.float32)
    for ci, (b, s) in enumerate(meta):
        hpv = res.results[ci]["hp"]          # [4, NL]
        aw = active[b]
        for t in range(len(aw)):
            heat[b, aw[t]] += hpv[t // NL, t % NL]
    out[:] = heat[:, None, :]
    return out
